# revision 53
# baseline (speedup 1.0000x reference)
"""Causal single-head attention (N=4096, D=F=1024) on 8 TRN2 NeuronCores.

Sequence-parallel with causal load balancing: core c owns TWO 256-row
query tiles {c, 15-c} (of 16), so every core does the same causal work:
band A attends <= 2048 keys, band B attends <= 4096 keys.

Two SPMD launches:
  A) QKV projection - each core computes q/k/v for its own 512 rows
     (weights replicated, host pre-transposed to contraction-major layouts).
  B) attention + output projection - each core consumes qT for its two
     query tiles plus per-band right-aligned key/value copies (kA: 16
     tiles, kB: 32 tiles; junk keys zeroed so exp(0)=1 probabilities
     multiply zeroed v rows and zeroed ones-columns and contribute
     nothing). The causal diagonal lands in the LAST two key tiles of
     each band regardless of core id, so the triangular mask is
     core-invariant and applied on-chip with affine_select.

Matmul operands are bf16 (f32 PSUM accumulation); scores/AV matmuls run
at free-dim 256 (one band's queries), projection at free-dim 512.
"""

import sys

try:
    import concourse.bass as bass
except ImportError:  # pragma: no cover
    sys.path.insert(0, "/opt/trn_rl_repo")
    import concourse.bass as bass

import ml_dtypes
import numpy as np

import concourse.mybir as mybir
import concourse.tile as tile
from concourse import bacc
from concourse.bass_utils import run_bass_kernel_spmd

N, D, F = 4096, 1024, 1024
C = 8              # cores
NL = N // C        # 512 query rows per core (2 bands x 256)
P = 128
SCALE = 1.0 / float(np.sqrt(np.float32(F)))

F32 = mybir.dt.float32
MM_DT = mybir.dt.bfloat16

DT = D // P        # 8 contraction tiles
FT = F // P        # 8 f tiles
MT = N // P        # 32 key tiles (band B)
MTA = 16           # key tiles band A
NT2 = NL // P      # 4 query-row tiles per core
BW = 256           # band width (queries per band)

LAST_EXEC_NS = [None, None]
LAST_RESULTS = [None, None]

_CACHE = {}


def _warmup(nc, pool, psum_pool, tag, width, n=10):
    warm = pool.tile([P, width], MM_DT)
    nc.vector.memset(warm, 0.0)
    wps = psum_pool.tile([P, width], F32, tag=tag)
    for wi in range(n):
        nc.tensor.matmul(wps, warm[:, :P], warm, start=(wi == 0), stop=(wi == n - 1))


def _build_qkv():
    nc = bacc.Bacc(None, target_bir_lowering=False)
    xT = nc.dram_tensor("xT", [P, DT, NL], MM_DT, kind="ExternalInput")
    wqb = nc.dram_tensor("wqb", [FT, P, DT, P], MM_DT, kind="ExternalInput")
    wkb = nc.dram_tensor("wkb", [FT, P, DT, P], MM_DT, kind="ExternalInput")
    wvb = nc.dram_tensor("wvb", [2, P, DT, 512], MM_DT, kind="ExternalInput")
    bq = nc.dram_tensor("bq", [P, FT], F32, kind="ExternalInput")
    bk = nc.dram_tensor("bk", [P, FT], F32, kind="ExternalInput")
    bvB = nc.dram_tensor("bvB", [P, F], F32, kind="ExternalInput")
    qT_o = nc.dram_tensor("qT_o", [F, NL], MM_DT, kind="ExternalOutput")
    kT_o = nc.dram_tensor("kT_o", [F, NL], MM_DT, kind="ExternalOutput")
    v_o = nc.dram_tensor("v_o", [NL, F], MM_DT, kind="ExternalOutput")

    with tile.TileContext(nc) as tc:
        with (
            tc.tile_pool(name="singles", bufs=1) as singles,
            tc.tile_pool(name="weights", bufs=8) as weights,
            tc.tile_pool(name="osb", bufs=6) as opool,
            tc.tile_pool(name="psum", bufs=8, space="PSUM") as psum,
        ):
            _warmup(nc, singles, psum, "ps", NL, n=12)
            xT_sb = singles.tile([P, DT, NL], MM_DT)
            nc.sync.dma_start(out=xT_sb, in_=xT.ap())
            bq_sb = singles.tile([P, FT], F32)
            nc.gpsimd.dma_start(out=bq_sb, in_=bq.ap())
            bk_sb = singles.tile([P, FT], F32)
            nc.gpsimd.dma_start(out=bk_sb, in_=bk.ap())
            bvB_sb = singles.tile([P, F], F32)
            nc.gpsimd.dma_start(out=bvB_sb, in_=bvB.ap())

            # q.T / k.T : out[f_tile, n] = sum_d wT[d, f] * xT[d, n]
            for w_t, b_sb, out_t in ((wqb, bq_sb, qT_o), (wkb, bk_sb, kT_o)):
                for ft in range(FT):
                    wc = weights.tile([P, DT, P], MM_DT, tag="wc")
                    nc.sync.dma_start(out=wc, in_=w_t.ap()[ft])
                    ps = psum.tile([P, NL], F32, tag="ps")
                    for dt_i in range(DT):
                        nc.tensor.matmul(
                            ps,
                            wc[:, dt_i, :],
                            xT_sb[:, dt_i, :],
                            start=(dt_i == 0),
                            stop=(dt_i == DT - 1),
                        )
                    osb = opool.tile([P, NL], MM_DT, tag="osb")
                    nc.vector.tensor_scalar_add(
                        out=osb, in0=ps, scalar1=b_sb[:, ft : ft + 1]
                    )
                    nc.scalar.dma_start(
                        out=out_t.ap()[ft * P : (ft + 1) * P, :], in_=osb
                    )

            # v : out[m_tile, f] = sum_d xT[d, m] * wvT[d, f]
            for fc in range(2):
                fs = slice(fc * 512, (fc + 1) * 512)
                wvc = weights.tile([P, DT, 512], MM_DT, tag="wvc")
                nc.sync.dma_start(out=wvc, in_=wvb.ap()[fc])
                for mi in range(NT2):
                    ps = psum.tile([P, 512], F32, tag="ps")
                    for dt_i in range(DT):
                        nc.tensor.matmul(
                            ps,
                            xT_sb[:, dt_i, mi * P : (mi + 1) * P],
                            wvc[:, dt_i, :],
                            start=(dt_i == 0),
                            stop=(dt_i == DT - 1),
                        )
                    vsb = opool.tile([P, 512], MM_DT, tag="osb")
                    nc.vector.tensor_add(out=vsb, in0=ps, in1=bvB_sb[:, fs])
                    nc.scalar.dma_start(
                        out=v_o.ap()[mi * P : (mi + 1) * P, fs], in_=vsb
                    )
    nc.finalize()
    return nc


def _build_attn():
    nc = bacc.Bacc(None, target_bir_lowering=False)
    qT = nc.dram_tensor("qT", [2, P, FT, BW], MM_DT, kind="ExternalInput")
    kAb = nc.dram_tensor("kAb", [MTA, P, FT, P], MM_DT, kind="ExternalInput")
    kBb = nc.dram_tensor("kBb", [MT, P, FT, P], MM_DT, kind="ExternalInput")
    vAb = nc.dram_tensor("vAb", [FT, P, MTA, P], MM_DT, kind="ExternalInput")
    vBb = nc.dram_tensor("vBb", [FT, 2, P, MTA, P], MM_DT, kind="ExternalInput")
    onesA = nc.dram_tensor("onesA", [P, MTA], MM_DT, kind="ExternalInput")
    onesB = nc.dram_tensor("onesB", [P, MT], MM_DT, kind="ExternalInput")
    projT = nc.dram_tensor("projT", [F, F], MM_DT, kind="ExternalInput")
    pbB = nc.dram_tensor("pbB", [P, F], F32, kind="ExternalInput")
    out_o = nc.dram_tensor("out_o", [NL, F], MM_DT, kind="ExternalOutput")

    with tile.TileContext(nc) as tc:
        with (
            tc.tile_pool(name="singles", bufs=1) as singles,
            tc.tile_pool(name="kc", bufs=20) as kpool,
            tc.tile_pool(name="pta", bufs=MTA) as ptapool,
            tc.tile_pool(name="ptb", bufs=MT) as ptbpool,
            tc.tile_pool(name="vc", bufs=6) as vpool,
            tc.tile_pool(name="osb", bufs=3) as opool,
            tc.tile_pool(name="sps", bufs=3, space="PSUM") as spsum,
            tc.tile_pool(name="rps", bufs=1, space="PSUM") as rpsum,
            tc.tile_pool(name="zps", bufs=2, space="PSUM") as zpsum,
            tc.tile_pool(name="ops", bufs=2, space="PSUM") as opsum,
            tc.tile_pool(name="dram", bufs=1, space="DRAM") as drampool,
        ):
            _warmup(nc, singles, opsum, "ops", BW, n=22)
            qTA_sb = singles.tile([P, FT, BW], MM_DT)
            nc.sync.dma_start(out=qTA_sb[:, :4, :], in_=qT.ap()[0, :, :4, :])
            nc.scalar.dma_start(out=qTA_sb[:, 4:, :], in_=qT.ap()[0, :, 4:, :])
            kc0 = kpool.tile([P, FT, P], MM_DT, tag="kc")
            nc.sync.dma_start(out=kc0[:, :4, :], in_=kAb.ap()[0, :, :4, :])
            nc.scalar.dma_start(out=kc0[:, 4:, :], in_=kAb.ap()[0, :, 4:, :])
            qTB_sb = singles.tile([P, FT, BW], MM_DT)
            onesA_sb = singles.tile([P, MTA], MM_DT)
            nc.gpsimd.dma_start(out=onesA_sb, in_=onesA.ap())
            onesB_sb = singles.tile([P, MT], MM_DT)
            nc.gpsimd.dma_start(out=onesB_sb, in_=onesB.ap())

            # ---- scores + exp per band: pT[m, n] = exp(SCALE * k.T q)
            def band_scores(ktensor, ntiles, qb_sb, ptpool, tag):
                pts = []
                for kt in range(ntiles):
                    if kt == 0 and ktensor is kAb:
                        kc = kc0
                    elif kt <= 5 and ktensor is kAb:
                        # ramp: split across both hardware queues so the
                        # first tiles land before the PE catches up
                        kc = kpool.tile([P, FT, P], MM_DT, tag="kc")
                        nc.sync.dma_start(
                            out=kc[:, :4, :], in_=ktensor.ap()[kt, :, :4, :]
                        )
                        nc.scalar.dma_start(
                            out=kc[:, 4:, :], in_=ktensor.ap()[kt, :, 4:, :]
                        )
                    else:
                        kc = kpool.tile([P, FT, P], MM_DT, tag="kc")
                        nc.sync.dma_start(out=kc, in_=ktensor.ap()[kt])
                    if kt == MTA - 1 and ktensor is kAb:
                        # qTB after band A's k tiles, ahead of band B's
                        nc.sync.dma_start(out=qTB_sb, in_=qT.ap()[1])
                    # zps pool is idle during scores: borrow its banks for
                    # a deeper effective psum rotation (3 sps + 2 zps)
                    if kt % 5 < 3:
                        ps = spsum.tile([P, BW], F32, tag="sps")
                    else:
                        ps = zpsum.tile([P, BW], F32, tag="zps")
                    for ft in range(FT):
                        nc.tensor.matmul(
                            ps,
                            kc[:, ft, :],
                            qb_sb[:, ft, :],
                            start=(ft == 0),
                            stop=(ft == FT - 1),
                        )
                    pt = ptpool.tile([P, BW], MM_DT, tag=tag)
                    nc.scalar.activation(
                        out=pt,
                        in_=ps,
                        func=mybir.ActivationFunctionType.Exp,
                        scale=SCALE,
                    )
                    if kt >= ntiles - 2:
                        # diagonal block: keep q >= key_local
                        nc.gpsimd.affine_select(
                            out=pt,
                            in_=pt,
                            pattern=[[1, BW]],
                            compare_op=mybir.AluOpType.is_ge,
                            fill=0.0,
                            base=-(kt - (ntiles - 2)) * P,
                            channel_multiplier=-1,
                        )
                    pts.append(pt)
                return pts

            ptsA = band_scores(kAb, MTA, qTA_sb, ptapool, "pta")
            ptsB = band_scores(kBb, MT, qTB_sb, ptbpool, "ptb")

            # ---- row sums per band (ones-column matmuls over valid keys)
            rpsA = rpsum.tile([1, BW], F32, tag="rps")
            for kt in range(MTA):
                nc.tensor.matmul(
                    rpsA,
                    onesA_sb[:, kt : kt + 1],
                    ptsA[kt],
                    start=(kt == 0),
                    stop=(kt == MTA - 1),
                )
            rpsB = rpsum.tile([1, BW], F32, tag="rps")
            for kt in range(MT):
                nc.tensor.matmul(
                    rpsB,
                    onesB_sb[:, kt : kt + 1],
                    ptsB[kt],
                    start=(kt == 0),
                    stop=(kt == MT - 1),
                )
            rs_row = singles.tile([1, NL], F32)
            nc.vector.tensor_copy(out=rs_row[:, :BW], in_=rpsA)
            nc.vector.tensor_copy(out=rs_row[:, BW:], in_=rpsB)
            scratch = drampool.tile([1, NL], F32)
            nc.sync.dma_start(out=scratch, in_=rs_row)
            rs_np = singles.tile([P, NT2], F32)
            nc.sync.dma_start(
                out=rs_np, in_=scratch[0].rearrange("(t p) -> p t", p=P)
            )
            recip_np = singles.tile([P, NT2], F32)
            nc.vector.reciprocal(out=recip_np, in_=rs_np)

            # ---- z.T[f, n] = sum_m v[m, f] * pT[m, n]  (per band)
            pbB_sb = singles.tile([P, F], F32)
            projT_sb = singles.tile([P, FT, F], MM_DT)
            z_tiles = []
            for ft in range(FT):
                if ft == 0:
                    nc.gpsimd.dma_start(out=pbB_sb, in_=pbB.ap())
                if ft == 1:
                    nc.gpsimd.dma_start(
                        out=projT_sb,
                        in_=projT.ap().rearrange("(t p) f -> p t f", p=P),
                    )
                zt = singles.tile([P, NL], MM_DT, tag=f"z{ft}")
                # band A
                vcA = vpool.tile([P, MTA, P], MM_DT, tag="vc")
                nc.scalar.dma_start(out=vcA, in_=vAb.ap()[ft])
                zpsA = zpsum.tile([P, BW], F32, tag="zps")
                for mi in range(MTA):
                    nc.tensor.matmul(
                        zpsA,
                        vcA[:, mi, :],
                        ptsA[mi],
                        start=(mi == 0),
                        stop=(mi == MTA - 1),
                    )
                nc.scalar.activation(
                    out=zt[:, :BW],
                    in_=zpsA,
                    func=mybir.ActivationFunctionType.Copy,
                )
                # band B (borrow scores/proj pools, idle during AV)
                if ft % 2 == 0:
                    zpsB = spsum.tile([P, BW], F32, tag="sps")
                else:
                    zpsB = opsum.tile([P, BW], F32, tag="ops")
                for vh in range(2):
                    vcB = vpool.tile([P, MTA, P], MM_DT, tag="vc")
                    nc.sync.dma_start(out=vcB, in_=vBb.ap()[ft, vh])
                    for mi in range(MTA):
                        mt = vh * MTA + mi
                        nc.tensor.matmul(
                            zpsB,
                            vcB[:, mi, :],
                            ptsB[mt],
                            start=(mt == 0),
                            stop=(mt == MT - 1),
                        )
                nc.vector.tensor_copy(out=zt[:, BW:], in_=zpsB)
                z_tiles.append(zt)

            # ---- out[n, o] = (z.T/rowsum) @ projT + pb
            for nt in range(NT2):
                for oc in range(2):
                    os_ = slice(oc * 512, (oc + 1) * 512)
                    ops = opsum.tile([P, 512], F32, tag="ops")
                    for ft in range(FT):
                        nc.tensor.matmul(
                            ops,
                            z_tiles[ft][:, nt * P : (nt + 1) * P],
                            projT_sb[:, ft, os_],
                            start=(ft == 0),
                            stop=(ft == FT - 1),
                        )
                    osb = opool.tile([P, 512], MM_DT, tag="osb")
                    nc.vector.scalar_tensor_tensor(
                        out=osb,
                        in0=ops,
                        scalar=recip_np[:, nt : nt + 1],
                        in1=pbB_sb[:, os_],
                        op0=mybir.AluOpType.mult,
                        op1=mybir.AluOpType.add,
                    )
                    nc.sync.dma_start(
                        out=out_o.ap()[nt * P : (nt + 1) * P, os_], in_=osb
                    )
    nc.finalize()
    return nc


def _get_programs():
    if "qkv" not in _CACHE:
        _CACHE["qkv"] = _build_qkv()
        _CACHE["attn"] = _build_attn()
    return _CACHE["qkv"], _CACHE["attn"]


def _c(a):
    return np.ascontiguousarray(a, dtype=np.float32)


def _b(a):
    return np.ascontiguousarray(np.asarray(a, dtype=np.float32).astype(ml_dtypes.bfloat16))


def kernel(x, wq_w, wq_b, wk_w, wk_b, wv_w, wv_b, proj_w, proj_b):
    x = np.asarray(x, dtype=np.float32)
    nc_qkv, nc_attn = _get_programs()

    # ---- launch A: QKV projection, sequence-sharded
    xT = np.asarray(x, dtype=np.float32).T        # [D, N]
    wqb = _b(np.asarray(wq_w).T.reshape(DT, P, FT, P).transpose(2, 1, 0, 3))
    wkb = _b(np.asarray(wk_w).T.reshape(DT, P, FT, P).transpose(2, 1, 0, 3))
    wvb = _b(np.asarray(wv_w).T.reshape(DT, P, 2, 512).transpose(2, 1, 0, 3))
    bq_pb = _c(np.asarray(wq_b).reshape(FT, P).T)   # [P, FT]
    bk_pb = _c(np.asarray(wk_b).reshape(FT, P).T)
    bvB = _c(np.broadcast_to(np.asarray(wv_b), (P, F)))
    in_a = []
    for c in range(C):
        xT_blk = _b(
            xT[:, c * NL : (c + 1) * NL].reshape(DT, P, NL).transpose(1, 0, 2)
        )
        in_a.append(
            {
                "xT": xT_blk,
                "wqb": wqb,
                "wkb": wkb,
                "wvb": wvb,
                "bq": bq_pb,
                "bk": bk_pb,
                "bvB": bvB,
            }
        )
    res_a = run_bass_kernel_spmd(nc_qkv, in_a, core_ids=list(range(C)))
    LAST_EXEC_NS[0] = res_a.exec_time_ns
    LAST_RESULTS[0] = res_a

    qT_full = np.concatenate([res_a.results[c]["qT_o"] for c in range(C)], axis=1)
    kT_full = np.concatenate([res_a.results[c]["kT_o"] for c in range(C)], axis=1)
    v_full = np.concatenate([res_a.results[c]["v_o"] for c in range(C)], axis=0)

    # ---- launch B: attention + projection (2-band causal balance)
    projT = _b(np.asarray(proj_w).T)              # [F, F]
    pbB = _c(np.broadcast_to(np.asarray(proj_b), (P, F)))
    in_b = []
    for c in range(C):
        LA = BW * (c + 1)            # valid keys band A
        LB = BW * (16 - c)           # valid keys band B
        tA, tB = c, 15 - c
        qT_blk = np.stack(
            [
                np.ascontiguousarray(
                    qT_full[:, t * BW : (t + 1) * BW]
                    .reshape(FT, P, BW)
                    .transpose(1, 0, 2)
                )
                for t in (tA, tB)
            ]
        )
        kA = np.zeros((F, MTA * P), dtype=ml_dtypes.bfloat16)
        kA[:, MTA * P - LA :] = kT_full[:, :LA]
        kAb = np.ascontiguousarray(
            kA.reshape(FT, P, MTA, P).transpose(2, 1, 0, 3)
        )
        kB = np.zeros((F, N), dtype=ml_dtypes.bfloat16)
        kB[:, N - LB :] = kT_full[:, :LB]
        kBb = np.ascontiguousarray(kB.reshape(FT, P, MT, P).transpose(2, 1, 0, 3))
        vA = np.zeros((MTA * P, F), dtype=ml_dtypes.bfloat16)
        vA[MTA * P - LA :, :] = v_full[:LA]
        vAb = np.ascontiguousarray(vA.reshape(MTA, P, FT, P).transpose(2, 1, 0, 3))
        vB = np.zeros((N, F), dtype=ml_dtypes.bfloat16)
        vB[N - LB :, :] = v_full[:LB]
        vBb = np.ascontiguousarray(
            vB.reshape(2, MTA, P, FT, P).transpose(3, 0, 2, 1, 4)
        )
        onesA_pb = np.zeros((P, MTA), dtype=ml_dtypes.bfloat16)
        onesA_pb[:, MTA - LA // P :] = 1.0
        onesB_pb = np.zeros((P, MT), dtype=ml_dtypes.bfloat16)
        onesB_pb[:, MT - LB // P :] = 1.0
        in_b.append(
            {
                "qT": qT_blk,
                "kAb": kAb,
                "kBb": kBb,
                "vAb": vAb,
                "vBb": vBb,
                "onesA": onesA_pb,
                "onesB": onesB_pb,
                "projT": projT,
                "pbB": pbB,
            }
        )
    res_b = run_bass_kernel_spmd(nc_attn, in_b, core_ids=list(range(C)))
    LAST_EXEC_NS[1] = res_b.exec_time_ns
    LAST_RESULTS[1] = res_b

    out = np.empty((N, F), dtype=np.float32)
    for c in range(C):
        tA, tB = c, 15 - c
        o = np.asarray(res_b.results[c]["out_o"], dtype=np.float32)
        out[tA * BW : (tA + 1) * BW] = o[:BW]
        out[tB * BW : (tB + 1) * BW] = o[BW:]
    return out


# revision 54
# speedup vs baseline: 1.1373x; 1.1373x over previous
"""Causal single-head attention (N=4096, D=F=1024) on 8 TRN2 NeuronCores.

Sequence-parallel with causal load balancing: core c owns TWO 256-row
query tiles {c, 15-c} (of 16), so every core does the same causal work:
band A attends <= 2048 keys, band B attends <= 4096 keys.

Two SPMD launches:
  A) QKV projection - each core computes q/k/v for its own 512 rows
     (weights replicated, host pre-transposed to contraction-major layouts).
  B) attention + output projection - each core consumes qT for its two
     query tiles plus per-band right-aligned key/value copies (kA: 16
     tiles, kB: 32 tiles; junk keys zeroed so exp(0)=1 probabilities
     multiply zeroed v rows and zeroed ones-columns and contribute
     nothing). The causal diagonal lands in the LAST two key tiles of
     each band regardless of core id, so the triangular mask is
     core-invariant and applied on-chip with affine_select.

Matmul operands are bf16 (f32 PSUM accumulation); scores/AV matmuls run
at free-dim 256 (one band's queries), projection at free-dim 512.
"""

import sys

try:
    import concourse.bass as bass
except ImportError:  # pragma: no cover
    sys.path.insert(0, "/opt/trn_rl_repo")
    import concourse.bass as bass

import ml_dtypes
import numpy as np

import concourse.mybir as mybir
import concourse.tile as tile
from concourse import bacc
from concourse.bass_utils import run_bass_kernel_spmd

N, D, F = 4096, 1024, 1024
C = 8              # cores
NL = N // C        # 512 query rows per core (2 bands x 256)
P = 128
SCALE = 1.0 / float(np.sqrt(np.float32(F)))

F32 = mybir.dt.float32
MM_DT = mybir.dt.bfloat16

DT = D // P        # 8 contraction tiles
FT = F // P        # 8 f tiles
MT = N // P        # 32 key tiles (band B)
MTA = 16           # key tiles band A
NT2 = NL // P      # 4 query-row tiles per core
BW = 256           # band width (queries per band)

LAST_EXEC_NS = [None, None]
LAST_RESULTS = [None, None]

_CACHE = {}


def _warmup(nc, pool, psum_pool, tag, width, n=10):
    warm = pool.tile([P, width], MM_DT)
    nc.vector.memset(warm, 0.0)
    wps = psum_pool.tile([P, width], F32, tag=tag)
    for wi in range(n):
        nc.tensor.matmul(wps, warm[:, :P], warm, start=(wi == 0), stop=(wi == n - 1))


def _build_qkv():
    nc = bacc.Bacc(None, target_bir_lowering=False)
    xT = nc.dram_tensor("xT", [P, DT, NL], MM_DT, kind="ExternalInput")
    wqb = nc.dram_tensor("wqb", [FT, P, DT, P], MM_DT, kind="ExternalInput")
    wkb = nc.dram_tensor("wkb", [FT, P, DT, P], MM_DT, kind="ExternalInput")
    wvb = nc.dram_tensor("wvb", [2, P, DT, 512], MM_DT, kind="ExternalInput")
    bq = nc.dram_tensor("bq", [P, FT], F32, kind="ExternalInput")
    bk = nc.dram_tensor("bk", [P, FT], F32, kind="ExternalInput")
    bvB = nc.dram_tensor("bvB", [P, F], F32, kind="ExternalInput")
    qT_o = nc.dram_tensor("qT_o", [F, NL], MM_DT, kind="ExternalOutput")
    kT_o = nc.dram_tensor("kT_o", [F, NL], MM_DT, kind="ExternalOutput")
    v_o = nc.dram_tensor("v_o", [NL, F], MM_DT, kind="ExternalOutput")

    with tile.TileContext(nc) as tc:
        with (
            tc.tile_pool(name="singles", bufs=1) as singles,
            tc.tile_pool(name="weights", bufs=8) as weights,
            tc.tile_pool(name="osb", bufs=6) as opool,
            tc.tile_pool(name="psum", bufs=8, space="PSUM") as psum,
        ):
            _warmup(nc, singles, psum, "ps", NL, n=12)
            xT_sb = singles.tile([P, DT, NL], MM_DT)
            nc.sync.dma_start(out=xT_sb, in_=xT.ap())
            bq_sb = singles.tile([P, FT], F32)
            nc.gpsimd.dma_start(out=bq_sb, in_=bq.ap())
            bk_sb = singles.tile([P, FT], F32)
            nc.gpsimd.dma_start(out=bk_sb, in_=bk.ap())
            bvB_sb = singles.tile([P, F], F32)
            nc.gpsimd.dma_start(out=bvB_sb, in_=bvB.ap())

            # q.T / k.T : out[f_tile, n] = sum_d wT[d, f] * xT[d, n]
            for w_t, b_sb, out_t in ((wqb, bq_sb, qT_o), (wkb, bk_sb, kT_o)):
                for ft in range(FT):
                    wc = weights.tile([P, DT, P], MM_DT, tag="wc")
                    nc.sync.dma_start(out=wc, in_=w_t.ap()[ft])
                    ps = psum.tile([P, NL], F32, tag="ps")
                    for dt_i in range(DT):
                        nc.tensor.matmul(
                            ps,
                            wc[:, dt_i, :],
                            xT_sb[:, dt_i, :],
                            start=(dt_i == 0),
                            stop=(dt_i == DT - 1),
                        )
                    osb = opool.tile([P, NL], MM_DT, tag="osb")
                    nc.vector.tensor_scalar_add(
                        out=osb, in0=ps, scalar1=b_sb[:, ft : ft + 1]
                    )
                    nc.scalar.dma_start(
                        out=out_t.ap()[ft * P : (ft + 1) * P, :], in_=osb
                    )

            # v : out[m_tile, f] = sum_d xT[d, m] * wvT[d, f]
            for fc in range(2):
                fs = slice(fc * 512, (fc + 1) * 512)
                wvc = weights.tile([P, DT, 512], MM_DT, tag="wvc")
                nc.sync.dma_start(out=wvc, in_=wvb.ap()[fc])
                for mi in range(NT2):
                    ps = psum.tile([P, 512], F32, tag="ps")
                    for dt_i in range(DT):
                        nc.tensor.matmul(
                            ps,
                            xT_sb[:, dt_i, mi * P : (mi + 1) * P],
                            wvc[:, dt_i, :],
                            start=(dt_i == 0),
                            stop=(dt_i == DT - 1),
                        )
                    vsb = opool.tile([P, 512], MM_DT, tag="osb")
                    nc.vector.tensor_add(out=vsb, in0=ps, in1=bvB_sb[:, fs])
                    nc.scalar.dma_start(
                        out=v_o.ap()[mi * P : (mi + 1) * P, fs], in_=vsb
                    )
    nc.finalize()
    return nc


def _build_attn():
    nc = bacc.Bacc(None, target_bir_lowering=False)
    qT = nc.dram_tensor("qT", [2, P, FT, BW], MM_DT, kind="ExternalInput")
    kAb = nc.dram_tensor("kAb", [MTA, P, FT, P], MM_DT, kind="ExternalInput")
    kBb = nc.dram_tensor("kBb", [MT, P, FT, P], MM_DT, kind="ExternalInput")
    vAb = nc.dram_tensor("vAb", [FT, P, MTA, P], MM_DT, kind="ExternalInput")
    vBb = nc.dram_tensor("vBb", [FT, 2, P, MTA, P], MM_DT, kind="ExternalInput")
    onesA = nc.dram_tensor("onesA", [P, MTA], MM_DT, kind="ExternalInput")
    onesB = nc.dram_tensor("onesB", [P, MT], MM_DT, kind="ExternalInput")
    projT = nc.dram_tensor("projT", [F, F], MM_DT, kind="ExternalInput")
    pbB = nc.dram_tensor("pbB", [P, F], F32, kind="ExternalInput")
    out_o = nc.dram_tensor("out_o", [NL, F], MM_DT, kind="ExternalOutput")

    with tile.TileContext(nc) as tc:
        with (
            tc.tile_pool(name="singles", bufs=1) as singles,
            tc.tile_pool(name="kc", bufs=26) as kpool,
            tc.tile_pool(name="pta", bufs=MTA) as ptapool,
            tc.tile_pool(name="ptb", bufs=MT) as ptbpool,
            tc.tile_pool(name="vc", bufs=8) as vpool,
            tc.tile_pool(name="osb", bufs=3) as opool,
            tc.tile_pool(name="sps", bufs=3, space="PSUM") as spsum,
            tc.tile_pool(name="rps", bufs=1, space="PSUM") as rpsum,
            tc.tile_pool(name="zps", bufs=2, space="PSUM") as zpsum,
            tc.tile_pool(name="ops", bufs=2, space="PSUM") as opsum,
            tc.tile_pool(name="dram", bufs=1, space="DRAM") as drampool,
        ):
            _warmup(nc, singles, opsum, "ops", BW, n=22)
            qTA_sb = singles.tile([P, FT, BW], MM_DT)
            nc.sync.dma_start(out=qTA_sb[:, :4, :], in_=qT.ap()[0, :, :4, :])
            nc.scalar.dma_start(out=qTA_sb[:, 4:, :], in_=qT.ap()[0, :, 4:, :])
            kc0 = kpool.tile([P, FT, P], MM_DT, tag="kc")
            nc.sync.dma_start(out=kc0[:, :4, :], in_=kAb.ap()[0, :, :4, :])
            nc.scalar.dma_start(out=kc0[:, 4:, :], in_=kAb.ap()[0, :, 4:, :])
            qTB_sb = singles.tile([P, FT, BW], MM_DT)
            onesA_sb = singles.tile([P, MTA], MM_DT)
            nc.gpsimd.dma_start(out=onesA_sb, in_=onesA.ap())
            onesB_sb = singles.tile([P, MT], MM_DT)
            nc.gpsimd.dma_start(out=onesB_sb, in_=onesB.ap())

            # ---- scores + exp per band: pT[m, n] = exp(SCALE * k.T q)
            def band_scores(ktensor, ntiles, qb_sb, ptpool, tag):
                pts = []
                for kt in range(ntiles):
                    if kt == 0 and ktensor is kAb:
                        kc = kc0
                    elif kt <= 5 and ktensor is kAb:
                        # ramp: split across both hardware queues so the
                        # first tiles land before the PE catches up
                        kc = kpool.tile([P, FT, P], MM_DT, tag="kc")
                        nc.sync.dma_start(
                            out=kc[:, :4, :], in_=ktensor.ap()[kt, :, :4, :]
                        )
                        nc.scalar.dma_start(
                            out=kc[:, 4:, :], in_=ktensor.ap()[kt, :, 4:, :]
                        )
                    else:
                        kc = kpool.tile([P, FT, P], MM_DT, tag="kc")
                        nc.sync.dma_start(out=kc, in_=ktensor.ap()[kt])
                    if kt == MTA - 1 and ktensor is kAb:
                        # qTB after band A's k tiles, ahead of band B's
                        nc.sync.dma_start(out=qTB_sb, in_=qT.ap()[1])
                    # zps pool is idle during scores: borrow its banks for
                    # a deeper effective psum rotation (3 sps + 2 zps)
                    if kt % 5 < 3:
                        ps = spsum.tile([P, BW], F32, tag="sps")
                    else:
                        ps = zpsum.tile([P, BW], F32, tag="zps")
                    for ft in range(FT):
                        nc.tensor.matmul(
                            ps,
                            kc[:, ft, :],
                            qb_sb[:, ft, :],
                            start=(ft == 0),
                            stop=(ft == FT - 1),
                        )
                    pt = ptpool.tile([P, BW], MM_DT, tag=tag)
                    nc.scalar.activation(
                        out=pt,
                        in_=ps,
                        func=mybir.ActivationFunctionType.Exp,
                        scale=SCALE,
                    )
                    if kt >= ntiles - 2:
                        # diagonal block: keep q >= key_local
                        nc.gpsimd.affine_select(
                            out=pt,
                            in_=pt,
                            pattern=[[1, BW]],
                            compare_op=mybir.AluOpType.is_ge,
                            fill=0.0,
                            base=-(kt - (ntiles - 2)) * P,
                            channel_multiplier=-1,
                        )
                    pts.append(pt)
                return pts

            ptsA = band_scores(kAb, MTA, qTA_sb, ptapool, "pta")
            ptsB = band_scores(kBb, MT, qTB_sb, ptbpool, "ptb")

            # ---- row sums per band (ones-column matmuls over valid keys)
            rpsA = rpsum.tile([1, BW], F32, tag="rps")
            for kt in range(MTA):
                nc.tensor.matmul(
                    rpsA,
                    onesA_sb[:, kt : kt + 1],
                    ptsA[kt],
                    start=(kt == 0),
                    stop=(kt == MTA - 1),
                )
            rpsB = rpsum.tile([1, BW], F32, tag="rps")
            for kt in range(MT):
                nc.tensor.matmul(
                    rpsB,
                    onesB_sb[:, kt : kt + 1],
                    ptsB[kt],
                    start=(kt == 0),
                    stop=(kt == MT - 1),
                )
            rs_row = singles.tile([1, NL], F32)
            nc.vector.tensor_copy(out=rs_row[:, :BW], in_=rpsA)
            nc.vector.tensor_copy(out=rs_row[:, BW:], in_=rpsB)
            scratch = drampool.tile([1, NL], F32)
            nc.sync.dma_start(out=scratch, in_=rs_row)
            rs_np = singles.tile([P, NT2], F32)
            nc.sync.dma_start(
                out=rs_np, in_=scratch[0].rearrange("(t p) -> p t", p=P)
            )
            recip_np = singles.tile([P, NT2], F32)
            nc.vector.reciprocal(out=recip_np, in_=rs_np)

            # ---- z.T[f, n] = sum_m v[m, f] * pT[m, n]  (per band)
            pbB_sb = singles.tile([P, F], F32)
            projT_sb = singles.tile([P, FT, F], MM_DT)
            z_tiles = []
            for ft in range(FT):
                if ft == 0:
                    nc.gpsimd.dma_start(out=pbB_sb, in_=pbB.ap())
                if ft == 1:
                    nc.gpsimd.dma_start(
                        out=projT_sb,
                        in_=projT.ap().rearrange("(t p) f -> p t f", p=P),
                    )
                zt = singles.tile([P, NL], MM_DT, tag=f"z{ft}")
                # band A
                vcA = vpool.tile([P, MTA, P], MM_DT, tag="vc")
                nc.scalar.dma_start(out=vcA, in_=vAb.ap()[ft])
                zpsA = zpsum.tile([P, BW], F32, tag="zps")
                for mi in range(MTA):
                    nc.tensor.matmul(
                        zpsA,
                        vcA[:, mi, :],
                        ptsA[mi],
                        start=(mi == 0),
                        stop=(mi == MTA - 1),
                    )
                nc.scalar.activation(
                    out=zt[:, :BW],
                    in_=zpsA,
                    func=mybir.ActivationFunctionType.Copy,
                )
                # band B (borrow scores/proj pools, idle during AV)
                if ft % 2 == 0:
                    zpsB = spsum.tile([P, BW], F32, tag="sps")
                else:
                    zpsB = opsum.tile([P, BW], F32, tag="ops")
                for vh in range(2):
                    vcB = vpool.tile([P, MTA, P], MM_DT, tag="vc")
                    nc.sync.dma_start(out=vcB, in_=vBb.ap()[ft, vh])
                    for mi in range(MTA):
                        mt = vh * MTA + mi
                        nc.tensor.matmul(
                            zpsB,
                            vcB[:, mi, :],
                            ptsB[mt],
                            start=(mt == 0),
                            stop=(mt == MT - 1),
                        )
                nc.vector.tensor_copy(out=zt[:, BW:], in_=zpsB)
                z_tiles.append(zt)

            # ---- out[n, o] = (z.T/rowsum) @ projT + pb
            for nt in range(NT2):
                for oc in range(2):
                    os_ = slice(oc * 512, (oc + 1) * 512)
                    ops = opsum.tile([P, 512], F32, tag="ops")
                    for ft in range(FT):
                        nc.tensor.matmul(
                            ops,
                            z_tiles[ft][:, nt * P : (nt + 1) * P],
                            projT_sb[:, ft, os_],
                            start=(ft == 0),
                            stop=(ft == FT - 1),
                        )
                    osb = opool.tile([P, 512], MM_DT, tag="osb")
                    nc.vector.scalar_tensor_tensor(
                        out=osb,
                        in0=ops,
                        scalar=recip_np[:, nt : nt + 1],
                        in1=pbB_sb[:, os_],
                        op0=mybir.AluOpType.mult,
                        op1=mybir.AluOpType.add,
                    )
                    nc.sync.dma_start(
                        out=out_o.ap()[nt * P : (nt + 1) * P, os_], in_=osb
                    )
    nc.finalize()
    return nc


def _get_programs():
    if "qkv" not in _CACHE:
        _CACHE["qkv"] = _build_qkv()
        _CACHE["attn"] = _build_attn()
    return _CACHE["qkv"], _CACHE["attn"]


def _c(a):
    return np.ascontiguousarray(a, dtype=np.float32)


def _b(a):
    return np.ascontiguousarray(np.asarray(a, dtype=np.float32).astype(ml_dtypes.bfloat16))


def kernel(x, wq_w, wq_b, wk_w, wk_b, wv_w, wv_b, proj_w, proj_b):
    x = np.asarray(x, dtype=np.float32)
    nc_qkv, nc_attn = _get_programs()

    # ---- launch A: QKV projection, sequence-sharded
    xT = np.asarray(x, dtype=np.float32).T        # [D, N]
    wqb = _b(np.asarray(wq_w).T.reshape(DT, P, FT, P).transpose(2, 1, 0, 3))
    wkb = _b(np.asarray(wk_w).T.reshape(DT, P, FT, P).transpose(2, 1, 0, 3))
    wvb = _b(np.asarray(wv_w).T.reshape(DT, P, 2, 512).transpose(2, 1, 0, 3))
    bq_pb = _c(np.asarray(wq_b).reshape(FT, P).T)   # [P, FT]
    bk_pb = _c(np.asarray(wk_b).reshape(FT, P).T)
    bvB = _c(np.broadcast_to(np.asarray(wv_b), (P, F)))
    in_a = []
    for c in range(C):
        xT_blk = _b(
            xT[:, c * NL : (c + 1) * NL].reshape(DT, P, NL).transpose(1, 0, 2)
        )
        in_a.append(
            {
                "xT": xT_blk,
                "wqb": wqb,
                "wkb": wkb,
                "wvb": wvb,
                "bq": bq_pb,
                "bk": bk_pb,
                "bvB": bvB,
            }
        )
    res_a = run_bass_kernel_spmd(nc_qkv, in_a, core_ids=list(range(C)))
    LAST_EXEC_NS[0] = res_a.exec_time_ns
    LAST_RESULTS[0] = res_a

    qT_full = np.concatenate([res_a.results[c]["qT_o"] for c in range(C)], axis=1)
    kT_full = np.concatenate([res_a.results[c]["kT_o"] for c in range(C)], axis=1)
    v_full = np.concatenate([res_a.results[c]["v_o"] for c in range(C)], axis=0)

    # ---- launch B: attention + projection (2-band causal balance)
    projT = _b(np.asarray(proj_w).T)              # [F, F]
    pbB = _c(np.broadcast_to(np.asarray(proj_b), (P, F)))
    in_b = []
    for c in range(C):
        LA = BW * (c + 1)            # valid keys band A
        LB = BW * (16 - c)           # valid keys band B
        tA, tB = c, 15 - c
        qT_blk = np.stack(
            [
                np.ascontiguousarray(
                    qT_full[:, t * BW : (t + 1) * BW]
                    .reshape(FT, P, BW)
                    .transpose(1, 0, 2)
                )
                for t in (tA, tB)
            ]
        )
        kA = np.zeros((F, MTA * P), dtype=ml_dtypes.bfloat16)
        kA[:, MTA * P - LA :] = kT_full[:, :LA]
        kAb = np.ascontiguousarray(
            kA.reshape(FT, P, MTA, P).transpose(2, 1, 0, 3)
        )
        kB = np.zeros((F, N), dtype=ml_dtypes.bfloat16)
        kB[:, N - LB :] = kT_full[:, :LB]
        kBb = np.ascontiguousarray(kB.reshape(FT, P, MT, P).transpose(2, 1, 0, 3))
        vA = np.zeros((MTA * P, F), dtype=ml_dtypes.bfloat16)
        vA[MTA * P - LA :, :] = v_full[:LA]
        vAb = np.ascontiguousarray(vA.reshape(MTA, P, FT, P).transpose(2, 1, 0, 3))
        vB = np.zeros((N, F), dtype=ml_dtypes.bfloat16)
        vB[N - LB :, :] = v_full[:LB]
        vBb = np.ascontiguousarray(
            vB.reshape(2, MTA, P, FT, P).transpose(3, 0, 2, 1, 4)
        )
        onesA_pb = np.zeros((P, MTA), dtype=ml_dtypes.bfloat16)
        onesA_pb[:, MTA - LA // P :] = 1.0
        onesB_pb = np.zeros((P, MT), dtype=ml_dtypes.bfloat16)
        onesB_pb[:, MT - LB // P :] = 1.0
        in_b.append(
            {
                "qT": qT_blk,
                "kAb": kAb,
                "kBb": kBb,
                "vAb": vAb,
                "vBb": vBb,
                "onesA": onesA_pb,
                "onesB": onesB_pb,
                "projT": projT,
                "pbB": pbB,
            }
        )
    res_b = run_bass_kernel_spmd(nc_attn, in_b, core_ids=list(range(C)))
    LAST_EXEC_NS[1] = res_b.exec_time_ns
    LAST_RESULTS[1] = res_b

    out = np.empty((N, F), dtype=np.float32)
    for c in range(C):
        tA, tB = c, 15 - c
        o = np.asarray(res_b.results[c]["out_o"], dtype=np.float32)
        out[tA * BW : (tA + 1) * BW] = o[:BW]
        out[tB * BW : (tB + 1) * BW] = o[BW:]
    return out


# revision 55
# speedup vs baseline: 1.1487x; 1.0100x over previous
"""Causal single-head attention (N=4096, D=F=1024) on 8 TRN2 NeuronCores.

Sequence-parallel with causal load balancing: core c owns TWO 256-row
query tiles {c, 15-c} (of 16), so every core does the same causal work:
band A attends <= 2048 keys, band B attends <= 4096 keys.

Two SPMD launches:
  A) QKV projection - each core computes q/k/v for its own 512 rows
     (weights replicated, host pre-transposed to contraction-major layouts).
  B) attention + output projection - each core consumes qT for its two
     query tiles plus per-band right-aligned key/value copies (kA: 16
     tiles, kB: 32 tiles; junk keys zeroed so exp(0)=1 probabilities
     multiply zeroed v rows and zeroed ones-columns and contribute
     nothing). The causal diagonal lands in the LAST two key tiles of
     each band regardless of core id, so the triangular mask is
     core-invariant and applied on-chip with affine_select.

Matmul operands are bf16 (f32 PSUM accumulation); scores/AV matmuls run
at free-dim 256 (one band's queries), projection at free-dim 512.
"""

import sys

try:
    import concourse.bass as bass
except ImportError:  # pragma: no cover
    sys.path.insert(0, "/opt/trn_rl_repo")
    import concourse.bass as bass

import ml_dtypes
import numpy as np

import concourse.mybir as mybir
import concourse.tile as tile
from concourse import bacc
from concourse.bass_utils import run_bass_kernel_spmd

N, D, F = 4096, 1024, 1024
C = 8              # cores
NL = N // C        # 512 query rows per core (2 bands x 256)
P = 128
SCALE = 1.0 / float(np.sqrt(np.float32(F)))

F32 = mybir.dt.float32
MM_DT = mybir.dt.bfloat16

DT = D // P        # 8 contraction tiles
FT = F // P        # 8 f tiles
MT = N // P        # 32 key tiles (band B)
MTA = 16           # key tiles band A
NT2 = NL // P      # 4 query-row tiles per core
BW = 256           # band width (queries per band)

LAST_EXEC_NS = [None, None]
LAST_RESULTS = [None, None]

_CACHE = {}


def _warmup(nc, pool, psum_pool, tag, width, n=10):
    warm = pool.tile([P, width], MM_DT)
    nc.vector.memset(warm, 0.0)
    wps = psum_pool.tile([P, width], F32, tag=tag)
    for wi in range(n):
        nc.tensor.matmul(wps, warm[:, :P], warm, start=(wi == 0), stop=(wi == n - 1))


def _build_qkv():
    nc = bacc.Bacc(None, target_bir_lowering=False)
    xT = nc.dram_tensor("xT", [P, DT, NL], MM_DT, kind="ExternalInput")
    wqb = nc.dram_tensor("wqb", [FT, P, DT, P], MM_DT, kind="ExternalInput")
    wkb = nc.dram_tensor("wkb", [FT, P, DT, P], MM_DT, kind="ExternalInput")
    wvb = nc.dram_tensor("wvb", [2, P, DT, 512], MM_DT, kind="ExternalInput")
    bq = nc.dram_tensor("bq", [P, FT], F32, kind="ExternalInput")
    bk = nc.dram_tensor("bk", [P, FT], F32, kind="ExternalInput")
    bvB = nc.dram_tensor("bvB", [P, F], F32, kind="ExternalInput")
    qT_o = nc.dram_tensor("qT_o", [F, NL], MM_DT, kind="ExternalOutput")
    kT_o = nc.dram_tensor("kT_o", [F, NL], MM_DT, kind="ExternalOutput")
    v_o = nc.dram_tensor("v_o", [NL, F], MM_DT, kind="ExternalOutput")

    with tile.TileContext(nc) as tc:
        with (
            tc.tile_pool(name="singles", bufs=1) as singles,
            tc.tile_pool(name="weights", bufs=8) as weights,
            tc.tile_pool(name="osb", bufs=6) as opool,
            tc.tile_pool(name="psum", bufs=8, space="PSUM") as psum,
        ):
            _warmup(nc, singles, psum, "ps", NL, n=12)
            xT_sb = singles.tile([P, DT, NL], MM_DT)
            nc.sync.dma_start(out=xT_sb, in_=xT.ap())
            bq_sb = singles.tile([P, FT], F32)
            nc.gpsimd.dma_start(out=bq_sb, in_=bq.ap())
            bk_sb = singles.tile([P, FT], F32)
            nc.gpsimd.dma_start(out=bk_sb, in_=bk.ap())
            bvB_sb = singles.tile([P, F], F32)
            nc.gpsimd.dma_start(out=bvB_sb, in_=bvB.ap())

            # q.T / k.T : out[f_tile, n] = sum_d wT[d, f] * xT[d, n]
            for w_t, b_sb, out_t in ((wqb, bq_sb, qT_o), (wkb, bk_sb, kT_o)):
                for ft in range(FT):
                    wc = weights.tile([P, DT, P], MM_DT, tag="wc")
                    nc.sync.dma_start(out=wc, in_=w_t.ap()[ft])
                    ps = psum.tile([P, NL], F32, tag="ps")
                    for dt_i in range(DT):
                        nc.tensor.matmul(
                            ps,
                            wc[:, dt_i, :],
                            xT_sb[:, dt_i, :],
                            start=(dt_i == 0),
                            stop=(dt_i == DT - 1),
                        )
                    osb = opool.tile([P, NL], MM_DT, tag="osb")
                    nc.vector.tensor_scalar_add(
                        out=osb, in0=ps, scalar1=b_sb[:, ft : ft + 1]
                    )
                    nc.scalar.dma_start(
                        out=out_t.ap()[ft * P : (ft + 1) * P, :], in_=osb
                    )

            # v : out[m_tile, f] = sum_d xT[d, m] * wvT[d, f]
            for fc in range(2):
                fs = slice(fc * 512, (fc + 1) * 512)
                wvc = weights.tile([P, DT, 512], MM_DT, tag="wvc")
                nc.sync.dma_start(out=wvc, in_=wvb.ap()[fc])
                for mi in range(NT2):
                    ps = psum.tile([P, 512], F32, tag="ps")
                    for dt_i in range(DT):
                        nc.tensor.matmul(
                            ps,
                            xT_sb[:, dt_i, mi * P : (mi + 1) * P],
                            wvc[:, dt_i, :],
                            start=(dt_i == 0),
                            stop=(dt_i == DT - 1),
                        )
                    vsb = opool.tile([P, 512], MM_DT, tag="osb")
                    nc.vector.tensor_add(out=vsb, in0=ps, in1=bvB_sb[:, fs])
                    nc.scalar.dma_start(
                        out=v_o.ap()[mi * P : (mi + 1) * P, fs], in_=vsb
                    )
    nc.finalize()
    return nc


def _build_attn():
    nc = bacc.Bacc(None, target_bir_lowering=False)
    qT = nc.dram_tensor("qT", [2, P, FT, BW], MM_DT, kind="ExternalInput")
    kAb = nc.dram_tensor("kAb", [MTA, P, FT, P], MM_DT, kind="ExternalInput")
    kBb = nc.dram_tensor("kBb", [MT, P, FT, P], MM_DT, kind="ExternalInput")
    vAb = nc.dram_tensor("vAb", [FT, P, MTA, P], MM_DT, kind="ExternalInput")
    vBb = nc.dram_tensor("vBb", [FT, 2, P, MTA, P], MM_DT, kind="ExternalInput")
    onesA = nc.dram_tensor("onesA", [P, MTA], MM_DT, kind="ExternalInput")
    onesB = nc.dram_tensor("onesB", [P, MT], MM_DT, kind="ExternalInput")
    projT = nc.dram_tensor("projT", [F, F], MM_DT, kind="ExternalInput")
    pbB = nc.dram_tensor("pbB", [P, F], F32, kind="ExternalInput")
    out_o = nc.dram_tensor("out_o", [NL, F], MM_DT, kind="ExternalOutput")

    with tile.TileContext(nc) as tc:
        with (
            tc.tile_pool(name="singles", bufs=1) as singles,
            tc.tile_pool(name="kc", bufs=20) as kpool,
            tc.tile_pool(name="pta", bufs=MTA) as ptapool,
            tc.tile_pool(name="ptb", bufs=MT) as ptbpool,
            tc.tile_pool(name="vc", bufs=6) as vpool,
            tc.tile_pool(name="osb", bufs=3) as opool,
            tc.tile_pool(name="sps", bufs=3, space="PSUM") as spsum,
            tc.tile_pool(name="rps", bufs=1, space="PSUM") as rpsum,
            tc.tile_pool(name="zps", bufs=2, space="PSUM") as zpsum,
            tc.tile_pool(name="ops", bufs=2, space="PSUM") as opsum,
            tc.tile_pool(name="dram", bufs=1, space="DRAM") as drampool,
        ):
            _warmup(nc, singles, opsum, "ops", BW, n=22)
            qTA_sb = singles.tile([P, FT, BW], MM_DT)
            nc.sync.dma_start(out=qTA_sb[:, :4, :], in_=qT.ap()[0, :, :4, :])
            nc.scalar.dma_start(out=qTA_sb[:, 4:, :], in_=qT.ap()[0, :, 4:, :])
            kc0 = kpool.tile([P, FT, P], MM_DT, tag="kc")
            nc.sync.dma_start(out=kc0[:, :4, :], in_=kAb.ap()[0, :, :4, :])
            nc.scalar.dma_start(out=kc0[:, 4:, :], in_=kAb.ap()[0, :, 4:, :])
            qTB_sb = singles.tile([P, FT, BW], MM_DT)
            onesA_sb = singles.tile([P, MTA], MM_DT)
            nc.gpsimd.dma_start(out=onesA_sb, in_=onesA.ap())
            onesB_sb = singles.tile([P, MT], MM_DT)
            nc.gpsimd.dma_start(out=onesB_sb, in_=onesB.ap())

            # ---- scores + exp per band: pT[m, n] = exp(SCALE * k.T q)
            def band_scores(ktensor, ntiles, qb_sb, ptpool, tag):
                pts = []
                for kt in range(ntiles):
                    if kt == 0 and ktensor is kAb:
                        kc = kc0
                    elif kt <= 5 and ktensor is kAb:
                        # ramp: split across both hardware queues so the
                        # first tiles land before the PE catches up
                        kc = kpool.tile([P, FT, P], MM_DT, tag="kc")
                        nc.sync.dma_start(
                            out=kc[:, :4, :], in_=ktensor.ap()[kt, :, :4, :]
                        )
                        nc.scalar.dma_start(
                            out=kc[:, 4:, :], in_=ktensor.ap()[kt, :, 4:, :]
                        )
                    else:
                        kc = kpool.tile([P, FT, P], MM_DT, tag="kc")
                        nc.sync.dma_start(out=kc, in_=ktensor.ap()[kt])
                    if kt == MTA - 1 and ktensor is kAb:
                        # qTB after band A's k tiles, ahead of band B's
                        nc.sync.dma_start(out=qTB_sb, in_=qT.ap()[1])
                    # zps pool is idle during scores: borrow its banks for
                    # a deeper effective psum rotation (3 sps + 2 zps)
                    if kt % 5 < 3:
                        ps = spsum.tile([P, BW], F32, tag="sps")
                    else:
                        ps = zpsum.tile([P, BW], F32, tag="zps")
                    for ft in range(FT):
                        nc.tensor.matmul(
                            ps,
                            kc[:, ft, :],
                            qb_sb[:, ft, :],
                            start=(ft == 0),
                            stop=(ft == FT - 1),
                        )
                    pt = ptpool.tile([P, BW], MM_DT, tag=tag)
                    nc.scalar.activation(
                        out=pt,
                        in_=ps,
                        func=mybir.ActivationFunctionType.Exp,
                        scale=SCALE,
                    )
                    if kt >= ntiles - 2:
                        # diagonal block: keep q >= key_local
                        nc.gpsimd.affine_select(
                            out=pt,
                            in_=pt,
                            pattern=[[1, BW]],
                            compare_op=mybir.AluOpType.is_ge,
                            fill=0.0,
                            base=-(kt - (ntiles - 2)) * P,
                            channel_multiplier=-1,
                        )
                    pts.append(pt)
                return pts

            ptsA = band_scores(kAb, MTA, qTA_sb, ptapool, "pta")
            ptsB = band_scores(kBb, MT, qTB_sb, ptbpool, "ptb")

            # ---- row sums per band (ones-column matmuls over valid keys)
            rpsA = rpsum.tile([1, BW], F32, tag="rps")
            for kt in range(MTA):
                nc.tensor.matmul(
                    rpsA,
                    onesA_sb[:, kt : kt + 1],
                    ptsA[kt],
                    start=(kt == 0),
                    stop=(kt == MTA - 1),
                )
            rpsB = rpsum.tile([1, BW], F32, tag="rps")
            for kt in range(MT):
                nc.tensor.matmul(
                    rpsB,
                    onesB_sb[:, kt : kt + 1],
                    ptsB[kt],
                    start=(kt == 0),
                    stop=(kt == MT - 1),
                )
            rs_row = singles.tile([1, NL], F32)
            nc.vector.tensor_copy(out=rs_row[:, :BW], in_=rpsA)
            nc.vector.tensor_copy(out=rs_row[:, BW:], in_=rpsB)
            scratch = drampool.tile([1, NL], F32)
            nc.sync.dma_start(out=scratch, in_=rs_row)
            rs_np = singles.tile([P, NT2], F32)
            nc.sync.dma_start(
                out=rs_np, in_=scratch[0].rearrange("(t p) -> p t", p=P)
            )
            recip_np = singles.tile([P, NT2], F32)
            nc.vector.reciprocal(out=recip_np, in_=rs_np)

            # ---- z.T[f, n] = sum_m v[m, f] * pT[m, n]  (per band)
            pbB_sb = singles.tile([P, F], F32)
            projT_sb = singles.tile([P, FT, F], MM_DT)
            z_tiles = []
            for ft in range(FT):
                if ft == 0:
                    nc.gpsimd.dma_start(out=pbB_sb, in_=pbB.ap())
                if ft == 1:
                    nc.gpsimd.dma_start(
                        out=projT_sb,
                        in_=projT.ap().rearrange("(t p) f -> p t f", p=P),
                    )
                zt = singles.tile([P, NL], MM_DT, tag=f"z{ft}")
                # band A
                vcA = vpool.tile([P, MTA, P], MM_DT, tag="vc")
                nc.scalar.dma_start(out=vcA, in_=vAb.ap()[ft])
                zpsA = zpsum.tile([P, BW], F32, tag="zps")
                for mi in range(MTA):
                    nc.tensor.matmul(
                        zpsA,
                        vcA[:, mi, :],
                        ptsA[mi],
                        start=(mi == 0),
                        stop=(mi == MTA - 1),
                    )
                nc.scalar.activation(
                    out=zt[:, :BW],
                    in_=zpsA,
                    func=mybir.ActivationFunctionType.Copy,
                )
                # band B (borrow scores/proj pools, idle during AV)
                if ft % 2 == 0:
                    zpsB = spsum.tile([P, BW], F32, tag="sps")
                else:
                    zpsB = opsum.tile([P, BW], F32, tag="ops")
                for vh in range(2):
                    vcB = vpool.tile([P, MTA, P], MM_DT, tag="vc")
                    nc.sync.dma_start(out=vcB, in_=vBb.ap()[ft, vh])
                    for mi in range(MTA):
                        mt = vh * MTA + mi
                        nc.tensor.matmul(
                            zpsB,
                            vcB[:, mi, :],
                            ptsB[mt],
                            start=(mt == 0),
                            stop=(mt == MT - 1),
                        )
                nc.vector.tensor_copy(out=zt[:, BW:], in_=zpsB)
                z_tiles.append(zt)

            # ---- out[n, o] = (z.T/rowsum) @ projT + pb
            for nt in range(NT2):
                for oc in range(2):
                    os_ = slice(oc * 512, (oc + 1) * 512)
                    ops = opsum.tile([P, 512], F32, tag="ops")
                    for ft in range(FT):
                        nc.tensor.matmul(
                            ops,
                            z_tiles[ft][:, nt * P : (nt + 1) * P],
                            projT_sb[:, ft, os_],
                            start=(ft == 0),
                            stop=(ft == FT - 1),
                        )
                    osb = opool.tile([P, 512], MM_DT, tag="osb")
                    nc.vector.scalar_tensor_tensor(
                        out=osb,
                        in0=ops,
                        scalar=recip_np[:, nt : nt + 1],
                        in1=pbB_sb[:, os_],
                        op0=mybir.AluOpType.mult,
                        op1=mybir.AluOpType.add,
                    )
                    nc.sync.dma_start(
                        out=out_o.ap()[nt * P : (nt + 1) * P, os_], in_=osb
                    )
    nc.finalize()
    return nc


def _get_programs():
    if "qkv" not in _CACHE:
        _CACHE["qkv"] = _build_qkv()
        _CACHE["attn"] = _build_attn()
    return _CACHE["qkv"], _CACHE["attn"]


def _c(a):
    return np.ascontiguousarray(a, dtype=np.float32)


def _b(a):
    return np.ascontiguousarray(np.asarray(a, dtype=np.float32).astype(ml_dtypes.bfloat16))


def kernel(x, wq_w, wq_b, wk_w, wk_b, wv_w, wv_b, proj_w, proj_b):
    x = np.asarray(x, dtype=np.float32)
    nc_qkv, nc_attn = _get_programs()

    # ---- launch A: QKV projection, sequence-sharded
    xT = np.asarray(x, dtype=np.float32).T        # [D, N]
    wqb = _b(np.asarray(wq_w).T.reshape(DT, P, FT, P).transpose(2, 1, 0, 3))
    wkb = _b(np.asarray(wk_w).T.reshape(DT, P, FT, P).transpose(2, 1, 0, 3))
    wvb = _b(np.asarray(wv_w).T.reshape(DT, P, 2, 512).transpose(2, 1, 0, 3))
    bq_pb = _c(np.asarray(wq_b).reshape(FT, P).T)   # [P, FT]
    bk_pb = _c(np.asarray(wk_b).reshape(FT, P).T)
    bvB = _c(np.broadcast_to(np.asarray(wv_b), (P, F)))
    in_a = []
    for c in range(C):
        xT_blk = _b(
            xT[:, c * NL : (c + 1) * NL].reshape(DT, P, NL).transpose(1, 0, 2)
        )
        in_a.append(
            {
                "xT": xT_blk,
                "wqb": wqb,
                "wkb": wkb,
                "wvb": wvb,
                "bq": bq_pb,
                "bk": bk_pb,
                "bvB": bvB,
            }
        )
    res_a = run_bass_kernel_spmd(nc_qkv, in_a, core_ids=list(range(C)))
    LAST_EXEC_NS[0] = res_a.exec_time_ns
    LAST_RESULTS[0] = res_a

    qT_full = np.concatenate([res_a.results[c]["qT_o"] for c in range(C)], axis=1)
    kT_full = np.concatenate([res_a.results[c]["kT_o"] for c in range(C)], axis=1)
    v_full = np.concatenate([res_a.results[c]["v_o"] for c in range(C)], axis=0)

    # ---- launch B: attention + projection (2-band causal balance)
    projT = _b(np.asarray(proj_w).T)              # [F, F]
    pbB = _c(np.broadcast_to(np.asarray(proj_b), (P, F)))
    in_b = []
    for c in range(C):
        LA = BW * (c + 1)            # valid keys band A
        LB = BW * (16 - c)           # valid keys band B
        tA, tB = c, 15 - c
        qT_blk = np.stack(
            [
                np.ascontiguousarray(
                    qT_full[:, t * BW : (t + 1) * BW]
                    .reshape(FT, P, BW)
                    .transpose(1, 0, 2)
                )
                for t in (tA, tB)
            ]
        )
        kA = np.zeros((F, MTA * P), dtype=ml_dtypes.bfloat16)
        kA[:, MTA * P - LA :] = kT_full[:, :LA]
        kAb = np.ascontiguousarray(
            kA.reshape(FT, P, MTA, P).transpose(2, 1, 0, 3)
        )
        kB = np.zeros((F, N), dtype=ml_dtypes.bfloat16)
        kB[:, N - LB :] = kT_full[:, :LB]
        kBb = np.ascontiguousarray(kB.reshape(FT, P, MT, P).transpose(2, 1, 0, 3))
        vA = np.zeros((MTA * P, F), dtype=ml_dtypes.bfloat16)
        vA[MTA * P - LA :, :] = v_full[:LA]
        vAb = np.ascontiguousarray(vA.reshape(MTA, P, FT, P).transpose(2, 1, 0, 3))
        vB = np.zeros((N, F), dtype=ml_dtypes.bfloat16)
        vB[N - LB :, :] = v_full[:LB]
        vBb = np.ascontiguousarray(
            vB.reshape(2, MTA, P, FT, P).transpose(3, 0, 2, 1, 4)
        )
        onesA_pb = np.zeros((P, MTA), dtype=ml_dtypes.bfloat16)
        onesA_pb[:, MTA - LA // P :] = 1.0
        onesB_pb = np.zeros((P, MT), dtype=ml_dtypes.bfloat16)
        onesB_pb[:, MT - LB // P :] = 1.0
        in_b.append(
            {
                "qT": qT_blk,
                "kAb": kAb,
                "kBb": kBb,
                "vAb": vAb,
                "vBb": vBb,
                "onesA": onesA_pb,
                "onesB": onesB_pb,
                "projT": projT,
                "pbB": pbB,
            }
        )
    res_b = run_bass_kernel_spmd(nc_attn, in_b, core_ids=list(range(C)))
    LAST_EXEC_NS[1] = res_b.exec_time_ns
    LAST_RESULTS[1] = res_b

    out = np.empty((N, F), dtype=np.float32)
    for c in range(C):
        tA, tB = c, 15 - c
        o = np.asarray(res_b.results[c]["out_o"], dtype=np.float32)
        out[tA * BW : (tA + 1) * BW] = o[:BW]
        out[tB * BW : (tB + 1) * BW] = o[BW:]
    return out
